# revision 11
# baseline (speedup 1.0000x reference)
"""Contrastive loss (InfoNCE, labels=arange) Trainium2 Bass kernel.

Problem: x, y [8192, 1024] f32.
  xn = l2norm(x); yn = l2norm(y)
  logits = xn @ yn.T / tau            [8192, 8192]
  loss = -mean(diag(log_softmax(logits)))

Strategy (8 NeuronCores, SPMD):
  - All prep runs on the host inside kernel(): l2-normalize x and y,
    scale by 16 and quantize to fp8e4m3, and lay the operands out
    pre-transposed + pre-interleaved for the PE's DoubleRow fp8 mode
    (2 fp8 weights per cell -> 256-deep contraction per instruction,
    ~1.4x bf16 throughput). The exact diagonal (unquantized) and the
    final log/mean also run on the host, so the device does ONLY:
    matmul -> exp (ScalarE, fused accumulate) -> tiny DMA out.
  - Data-parallel shard of x rows: core c computes the [1024, 8192]
    logits slab for x rows [c*1024, (c+1)*1024) against all of y.
    Both operands live in SBUF for the whole kernel (73 KB/partition).
  - The PE stream (512 DoubleRow matmuls x 216 ns) is the critical
    path; ScalarE drains each 2-bank PSUM tile (1024 logits) in
    ~1.4us vs the PE's ~1.7us fill, so exp+accumulate stays hidden.
  - Input DMAs are ordered by first use on one HWDGE queue (both
    queues share the 16 SDMA engines, ~320 GB/s aggregate): x m-tile
    0, the four k-chunks of y block 0, the rest of x, then y blocks
    1..7. First matmul starts after ~0.4 MB instead of ~2 MB.
  - No max-subtraction in softmax: |cos|/tau <= ~14.6 so exp stays in
    f32 range; the diagonal term is applied on the host from the exact
    (unquantized) normalized dot product.
  - fp8 error budget: elementwise quant noise ~2^-4 rel -> cosine noise
    ~1.6e-3 -> logit noise ~0.023, which averages out across 8192
    softmax terms (loss bias ~3e-5 rel; tolerance is 2e-2).
  - A short burst of dummy matmuls warms the PE's HAM clock gate
    (1.2 -> 2.4 GHz) while the input DMAs stream in.
"""

import numpy as np
import ml_dtypes

import concourse.bacc as bacc
import concourse.tile as tile
from concourse import mybir
from concourse.bass_utils import run_bass_kernel_spmd

B = 8192
D = 1024
N_CORES = 8
ROWS = B // N_CORES          # 1024 x-rows per core
MT = ROWS // 128             # 8 m-tiles per core
KB = D // 256                # 4 k-blocks of 256 (DoubleRow: 2x128 per matmul)
NB = 8                       # y column blocks
YBLK = B // NB               # 1024 y rows per block
TAU = 0.07
EPS = 1e-12                  # matches torch F.normalize eps
FP8_SCALE = 16.0             # keeps fp8 operands in normal range
ACT_SCALE = 1.0 / (FP8_SCALE * FP8_SCALE * TAU)
WARMUP_MM = 24               # ~2.6us of N=128 matmuls bridging the input DMA
                             # wait, so the HAM clock gate is open (2.4 GHz)
                             # when the real stream starts

F8 = mybir.dt.float8e4
BF16 = mybir.dt.bfloat16
F32 = mybir.dt.float32
AF = mybir.ActivationFunctionType
ALU = mybir.AluOpType

_compiled = None


def _build():
    nc = bacc.Bacc(
        "TRN2", target_bir_lowering=False, debug=False, num_devices=N_CORES
    )
    xT_d = nc.dram_tensor("xT", [MT, 128, KB, 2, 128], F8, kind="ExternalInput").ap()
    yT_d = nc.dram_tensor("yT", [NB, KB, 128, 2, YBLK], F8, kind="ExternalInput").ap()
    out_d = nc.dram_tensor("out", [128, MT, NB], F32, kind="ExternalOutput").ap()

    with tile.TileContext(nc) as tc:
        with (
            tc.tile_pool(name="persist", bufs=1) as persist,
            tc.tile_pool(name="psum", bufs=4, space="PSUM") as psum,
        ):
            xT = persist.tile([128, MT, KB, 2, 128], F8)
            yT = persist.tile([128, NB, KB, 2, YBLK], F8)
            sumexp = persist.tile([128, MT, NB], F32)
            warm = persist.tile([128, 128], BF16)

            # HAM warm-up first: PE busy during the input DMA so the clock
            # gate is already 8/8 when the real matmuls start.
            nc.gpsimd.memset(warm, 0.0)
            wps = psum.tile([128, 1024], F32, tag="ps", name="wps")
            for _ in range(WARMUP_MM):
                nc.tensor.matmul(
                    wps[:, 0:128], lhsT=warm, rhs=warm, start=True, stop=True
                )

            # Input DMAs on one HWDGE queue, ordered by first use.
            nc.sync.dma_start(out=xT[:, 0], in_=xT_d[0])
            for kb in range(KB):
                nc.sync.dma_start(out=yT[:, 0, kb], in_=yT_d[0, kb])
            for mi in range(1, MT):
                nc.sync.dma_start(out=xT[:, mi], in_=xT_d[mi])
            for nb in range(1, NB):
                nc.sync.dma_start(out=yT[:, nb], in_=yT_d[nb])

            # One 2-bank PSUM tile per (y block, m tile): 8 matmuls fill it
            # (~1.7us), one wide ACT drains it (~1.4us) - ScalarE hidden.
            for nb in range(NB):
                for mi in range(MT):
                    ps = psum.tile([128, 1024], F32, tag="ps", name="ps")
                    for nh in range(2):
                        c0 = nh * 512
                        for kb in range(KB):
                            nc.tensor.matmul(
                                ps[:, c0:c0 + 512],
                                lhsT=xT[:, mi, kb],
                                rhs=yT[:, nb, kb, :, c0:c0 + 512],
                                start=(kb == 0),
                                stop=(kb == KB - 1),
                                perf_mode=mybir.MatmulPerfMode.DoubleRow,
                            )
                    nc.scalar.activation(
                        out=ps, in_=ps, func=AF.Exp, scale=ACT_SCALE,
                        accum_out=sumexp[:, mi, nb:nb + 1],
                    )
                # ship this block's sums while the next block computes
                nc.scalar.dma_start(out=out_d[:, :, nb], in_=sumexp[:, :, nb])

    nc.compile()
    return nc


def _prep(x: np.ndarray, y: np.ndarray):
    """Host prep: normalize, fp8-quantize, PE-layout both operands."""
    x = np.ascontiguousarray(x, dtype=np.float32)
    y = np.ascontiguousarray(y, dtype=np.float32)
    xn = x / np.maximum(np.linalg.norm(x, axis=1, keepdims=True), EPS)
    yn = y / np.maximum(np.linalg.norm(y, axis=1, keepdims=True), EPS)
    diag = np.einsum("ij,ij->i", xn.astype(np.float64), yn.astype(np.float64))

    f8 = ml_dtypes.float8_e4m3
    xq = (xn * FP8_SCALE).astype(f8)
    yq = (yn * FP8_SCALE).astype(f8)

    # xT[c, mi, p, kb, i, m] = xq[c*1024 + mi*128 + m, kb*256 + i*128 + p]
    xT = np.ascontiguousarray(
        xq.reshape(N_CORES, MT, 128, KB, 2, 128).transpose(0, 1, 5, 3, 4, 2)
    )
    # yT[nb, kb, p, i, n] = yq[nb*1024 + n, kb*256 + i*128 + p]
    yT = np.ascontiguousarray(
        yq.reshape(NB, YBLK, KB, 2, 128).transpose(0, 2, 4, 3, 1)
    )
    return xT, yT, diag


def _finalize(res, diag) -> np.ndarray:
    total = 0.0
    for c in range(N_CORES):
        S = res.results[c]["out"].astype(np.float64).sum(axis=2)  # [p, mi]
        dg = diag[c * ROWS:(c + 1) * ROWS].reshape(MT, 128)       # [mi, p]
        total += (np.log(S.T) - dg / TAU).sum()
    return np.float32(total / B)


def kernel(x: np.ndarray, y: np.ndarray) -> np.ndarray:
    global _compiled
    if _compiled is None:
        _compiled = _build()
    nc = _compiled

    xT, yT, diag = _prep(x, y)
    in_maps = [{"xT": xT[c], "yT": yT} for c in range(N_CORES)]
    res = run_bass_kernel_spmd(nc, in_maps, core_ids=list(range(N_CORES)))
    return _finalize(res, diag)


# revision 12
# speedup vs baseline: 1.1661x; 1.1661x over previous
"""Contrastive loss (InfoNCE, labels=arange) Trainium2 Bass kernel.

Problem: x, y [8192, 1024] f32.
  xn = l2norm(x); yn = l2norm(y)
  logits = xn @ yn.T / tau            [8192, 8192]
  loss = -mean(diag(log_softmax(logits)))

Strategy (8 NeuronCores, SPMD):
  - All prep runs on the host inside kernel(): l2-normalize x and y,
    scale by 16 and quantize to fp8e4m3, and lay the operands out
    pre-transposed + pre-interleaved for the PE's DoubleRow fp8 mode
    (2 fp8 weights per cell -> 256-deep contraction per instruction,
    ~1.4x bf16 throughput). The exact diagonal (unquantized) and the
    final log/mean also run on the host, so the device does ONLY:
    matmul -> exp (ScalarE, fused accumulate) -> tiny DMA out.
  - Data-parallel shard of x rows: core c computes the [1024, 8192]
    logits slab for x rows [c*1024, (c+1)*1024) against all of y.
    Both operands live in SBUF for the whole kernel (73 KB/partition).
  - The PE stream (512 DoubleRow matmuls x 216 ns) is the critical
    path; ScalarE drains each 2-bank PSUM tile (1024 logits) in
    ~1.4us vs the PE's ~1.7us fill, so exp+accumulate stays hidden.
  - Input DMAs are ordered by first use on one HWDGE queue (both
    queues share the 16 SDMA engines, ~320 GB/s aggregate): x m-tile
    0, the four k-chunks of y block 0, the rest of x, then y blocks
    1..7. First matmul starts after ~0.4 MB instead of ~2 MB.
  - No max-subtraction in softmax: |cos|/tau <= ~14.6 so exp stays in
    f32 range; the diagonal term is applied on the host from the exact
    (unquantized) normalized dot product.
  - fp8 error budget: elementwise quant noise ~2^-4 rel -> cosine noise
    ~1.6e-3 -> logit noise ~0.023, which averages out across 8192
    softmax terms (loss bias ~3e-5 rel; tolerance is 2e-2).
  - A short burst of dummy matmuls warms the PE's HAM clock gate
    (1.2 -> 2.4 GHz) while the input DMAs stream in.
"""

import numpy as np
import ml_dtypes

import concourse.bacc as bacc
import concourse.tile as tile
from concourse import mybir
from concourse.bass_utils import run_bass_kernel_spmd

B = 8192
D = 1024
N_CORES = 8
ROWS = B // N_CORES          # 1024 x-rows per core
MT = ROWS // 128             # 8 m-tiles per core
KB = D // 256                # 4 k-blocks of 256 (DoubleRow: 2x128 per matmul)
NB = 8                       # y column blocks
YBLK = B // NB               # 1024 y rows per block
TAU = 0.07
EPS = 1e-12                  # matches torch F.normalize eps
FP8_SCALE = 16.0             # keeps fp8 operands in normal range
ACT_SCALE = 1.0 / (FP8_SCALE * FP8_SCALE * TAU)
WARMUP_MM = 24               # ~2.6us of N=128 matmuls bridging the input DMA
                             # wait, so the HAM clock gate is open (2.4 GHz)
                             # when the real stream starts

F8 = mybir.dt.float8e4
BF16 = mybir.dt.bfloat16
F32 = mybir.dt.float32
AF = mybir.ActivationFunctionType
ALU = mybir.AluOpType

_compiled = None


def _build():
    nc = bacc.Bacc(
        "TRN2", target_bir_lowering=False, debug=False, num_devices=N_CORES
    )
    xT_d = nc.dram_tensor("xT", [MT, 128, KB, 2, 128], F8, kind="ExternalInput").ap()
    yT_d = nc.dram_tensor("yT", [NB, KB, 128, 2, YBLK], F8, kind="ExternalInput").ap()
    NP2 = NB // 2
    out_d = nc.dram_tensor("out", [128, MT, NP2], F32, kind="ExternalOutput").ap()

    with tile.TileContext(nc) as tc:
        with (
            tc.tile_pool(name="persist", bufs=1) as persist,
            tc.tile_pool(name="psum", bufs=2, space="PSUM") as psum,
        ):
            xT = persist.tile([128, MT, KB, 2, 128], F8)
            yT = persist.tile([128, NB, KB, 2, YBLK], F8)
            sumexp = persist.tile([128, MT, NP2], F32)
            warm = persist.tile([128, 128], BF16)

            # HAM warm-up first: PE busy during the input DMA so the clock
            # gate is already 8/8 when the real matmuls start.
            nc.gpsimd.memset(warm, 0.0)
            wps = psum.tile([128, 2048], F32, tag="ps", name="wps")
            for _ in range(WARMUP_MM):
                nc.tensor.matmul(
                    wps[:, 0:128], lhsT=warm, rhs=warm, start=True, stop=True
                )

            # Input DMAs on one HWDGE queue, chunked and ordered by first
            # use so the PE stream starts after ~0.4 MB, not ~2 MB.
            nc.sync.dma_start(out=xT[:, 0], in_=xT_d[0])
            for kb in range(KB):
                nc.sync.dma_start(out=yT[:, 0, kb], in_=yT_d[0, kb])
            nc.sync.dma_start(out=xT[:, 1], in_=xT_d[1])
            for kb in range(KB):
                nc.sync.dma_start(out=yT[:, 1, kb], in_=yT_d[1, kb])
            nc.sync.dma_start(out=xT[:, 2:], in_=xT_d[2:])
            for nb in range(2, NB):
                for kb in range(KB):
                    nc.sync.dma_start(out=yT[:, nb, kb], in_=yT_d[nb, kb])

            # One 4-bank PSUM tile per (y block pair, m tile): 16 matmuls
            # fill it (~3.5us), one wide ACT drains it (~2.2us) so ScalarE
            # stays well off the critical path.
            for np2 in range(NP2):
                for mi in range(MT):
                    ps = psum.tile([128, 2048], F32, tag="ps", name="ps")
                    for nh in range(4):
                        nb = np2 * 2 + nh // 2
                        c0 = (nh % 2) * 512
                        for kb in range(KB):
                            nc.tensor.matmul(
                                ps[:, nh * 512:(nh + 1) * 512],
                                lhsT=xT[:, mi, kb],
                                rhs=yT[:, nb, kb, :, c0:c0 + 512],
                                start=(kb == 0),
                                stop=(kb == KB - 1),
                                perf_mode=mybir.MatmulPerfMode.DoubleRow,
                            )
                    nc.scalar.activation(
                        out=ps, in_=ps, func=AF.Exp, scale=ACT_SCALE,
                        accum_out=sumexp[:, mi, np2:np2 + 1],
                    )
                # ship this block pair's sums while the next pair computes
                nc.scalar.dma_start(out=out_d[:, :, np2], in_=sumexp[:, :, np2])

    nc.compile()
    return nc


def _prep(x: np.ndarray, y: np.ndarray):
    """Host prep: normalize, fp8-quantize, PE-layout both operands."""
    x = np.ascontiguousarray(x, dtype=np.float32)
    y = np.ascontiguousarray(y, dtype=np.float32)
    xn = x / np.maximum(np.linalg.norm(x, axis=1, keepdims=True), EPS)
    yn = y / np.maximum(np.linalg.norm(y, axis=1, keepdims=True), EPS)
    diag = np.einsum("ij,ij->i", xn.astype(np.float64), yn.astype(np.float64))

    f8 = ml_dtypes.float8_e4m3
    xq = (xn * FP8_SCALE).astype(f8)
    yq = (yn * FP8_SCALE).astype(f8)

    # xT[c, mi, p, kb, i, m] = xq[c*1024 + mi*128 + m, kb*256 + i*128 + p]
    xT = np.ascontiguousarray(
        xq.reshape(N_CORES, MT, 128, KB, 2, 128).transpose(0, 1, 5, 3, 4, 2)
    )
    # yT[nb, kb, p, i, n] = yq[nb*1024 + n, kb*256 + i*128 + p]
    yT = np.ascontiguousarray(
        yq.reshape(NB, YBLK, KB, 2, 128).transpose(0, 2, 4, 3, 1)
    )
    return xT, yT, diag


def _finalize(res, diag) -> np.ndarray:
    total = 0.0
    for c in range(N_CORES):
        S = res.results[c]["out"].astype(np.float64).sum(axis=2)  # [p, mi]
        dg = diag[c * ROWS:(c + 1) * ROWS].reshape(MT, 128)       # [mi, p]
        total += (np.log(S.T) - dg / TAU).sum()
    return np.float32(total / B)


def kernel(x: np.ndarray, y: np.ndarray) -> np.ndarray:
    global _compiled
    if _compiled is None:
        _compiled = _build()
    nc = _compiled

    xT, yT, diag = _prep(x, y)
    in_maps = [{"xT": xT[c], "yT": yT} for c in range(N_CORES)]
    res = run_bass_kernel_spmd(nc, in_maps, core_ids=list(range(N_CORES)))
    return _finalize(res, diag)


# revision 16
# speedup vs baseline: 1.1942x; 1.0241x over previous
"""Contrastive loss (InfoNCE, labels=arange) Trainium2 Bass kernel.

Problem: x, y [8192, 1024] f32.
  xn = l2norm(x); yn = l2norm(y)
  logits = xn @ yn.T / tau            [8192, 8192]
  loss = -mean(diag(log_softmax(logits)))

Strategy (8 NeuronCores, SPMD):
  - All prep runs on the host inside kernel(): l2-normalize x and y,
    scale by 16 and quantize to fp8e4m3, and lay the operands out
    pre-transposed + pre-interleaved for the PE's DoubleRow fp8 mode
    (2 fp8 weights per cell -> 256-deep contraction per instruction,
    ~1.4x bf16 throughput). The exact diagonal (unquantized) and the
    final log/mean also run on the host, so the device does ONLY:
    matmul -> exp (ScalarE, fused accumulate) -> tiny DMA out.
  - Data-parallel shard of x rows: core c computes the [1024, 8192]
    logits slab for x rows [c*1024, (c+1)*1024) against all of y.
    Both operands live in SBUF for the whole kernel (73 KB/partition).
  - The PE stream (512 DoubleRow matmuls x 216 ns) is the critical
    path; ScalarE drains each 2-bank PSUM tile (1024 logits) in
    ~1.4us vs the PE's ~1.7us fill, so exp+accumulate stays hidden.
  - Input DMAs are ordered by first use on one HWDGE queue (both
    queues share the 16 SDMA engines, ~320 GB/s aggregate): x m-tile
    0, the four k-chunks of y block 0, the rest of x, then y blocks
    1..7. First matmul starts after ~0.4 MB instead of ~2 MB.
  - No max-subtraction in softmax: |cos|/tau <= ~14.6 so exp stays in
    f32 range; the diagonal term is applied on the host from the exact
    (unquantized) normalized dot product.
  - fp8 error budget: elementwise quant noise ~2^-4 rel -> cosine noise
    ~1.6e-3 -> logit noise ~0.023, which averages out across 8192
    softmax terms (loss bias ~3e-5 rel; tolerance is 2e-2).
  - A short burst of dummy matmuls warms the PE's HAM clock gate
    (1.2 -> 2.4 GHz) while the input DMAs stream in.
"""

import numpy as np
import ml_dtypes

import concourse.bacc as bacc
import concourse.tile as tile
from concourse import mybir
from concourse.bass_utils import run_bass_kernel_spmd

B = 8192
D = 1024
N_CORES = 8
ROWS = B // N_CORES          # 1024 x-rows per core
MT = ROWS // 128             # 8 m-tiles per core
KB = D // 256                # 4 k-blocks of 256 (DoubleRow: 2x128 per matmul)
NB = 8                       # y column blocks
YBLK = B // NB               # 1024 y rows per block
TAU = 0.07
EPS = 1e-12                  # matches torch F.normalize eps
FP8_SCALE = 16.0             # keeps fp8 operands in normal range
ACT_SCALE = 1.0 / (FP8_SCALE * FP8_SCALE * TAU)
WARMUP_MM = 24               # ~2.6us of N=128 matmuls bridging the input DMA
                             # wait, so the HAM clock gate is open (2.4 GHz)
                             # when the real stream starts

F8 = mybir.dt.float8e4
BF16 = mybir.dt.bfloat16
F32 = mybir.dt.float32
AF = mybir.ActivationFunctionType
ALU = mybir.AluOpType

_compiled = None


def _build():
    nc = bacc.Bacc(
        "TRN2", target_bir_lowering=False, debug=False, num_devices=N_CORES
    )
    xT_d = nc.dram_tensor("xT", [MT, 128, KB, 2, 128], F8, kind="ExternalInput").ap()
    yT_d = nc.dram_tensor("yT", [NB, KB, 128, 2, YBLK], F8, kind="ExternalInput").ap()
    NP2 = NB // 2
    out_d = nc.dram_tensor("out", [128, NP2, MT], F32, kind="ExternalOutput").ap()

    with tile.TileContext(nc) as tc:
        with (
            tc.tile_pool(name="persist", bufs=1) as persist,
            tc.tile_pool(name="psum", bufs=2, space="PSUM") as psum,
        ):
            xT = persist.tile([128, MT, KB, 2, 128], F8)
            yT = persist.tile([128, NB, KB, 2, YBLK], F8)
            sumexp = persist.tile([128, NP2, MT], F32)
            warm = persist.tile([128, 128], BF16)

            # HAM warm-up first: PE busy during the input DMA so the clock
            # gate is already 8/8 when the real matmuls start.
            nc.gpsimd.memset(warm, 0.0)
            wps = psum.tile([128, 2048], F32, tag="ps", name="wps")
            for _ in range(WARMUP_MM):
                nc.tensor.matmul(
                    wps[:, 0:128], lhsT=warm, rhs=warm, start=True, stop=True
                )

            # Input DMAs on one HWDGE queue, chunked and ordered by first
            # use so the PE stream starts after ~0.4 MB, not ~2 MB.
            nc.sync.dma_start(out=xT[:, 0], in_=xT_d[0])
            for kb in range(KB):
                nc.sync.dma_start(out=yT[:, 0, kb], in_=yT_d[0, kb])
            nc.sync.dma_start(out=xT[:, 1], in_=xT_d[1])
            for kb in range(KB):
                nc.sync.dma_start(out=yT[:, 1, kb], in_=yT_d[1, kb])
            nc.sync.dma_start(out=xT[:, 2:], in_=xT_d[2:])
            for nb in range(2, NB):
                for kb in range(KB):
                    nc.sync.dma_start(out=yT[:, nb, kb], in_=yT_d[nb, kb])

            # One 4-bank PSUM tile per (y block pair, m tile): 16 matmuls
            # fill it (~3.5us), one wide ACT drains it (~2.2us) so ScalarE
            # stays well off the critical path.
            for np2 in range(NP2):
                for mi in range(MT):
                    ps = psum.tile([128, 2048], F32, tag="ps", name="ps")
                    for nh in range(4):
                        nb = np2 * 2 + nh // 2
                        c0 = (nh % 2) * 512
                        for kb in range(KB):
                            nc.tensor.matmul(
                                ps[:, nh * 512:(nh + 1) * 512],
                                lhsT=xT[:, mi, kb],
                                rhs=yT[:, nb, kb, :, c0:c0 + 512],
                                start=(kb == 0),
                                stop=(kb == KB - 1),
                                perf_mode=mybir.MatmulPerfMode.DoubleRow,
                            )
                    nc.scalar.activation(
                        out=ps, in_=ps, func=AF.Exp, scale=ACT_SCALE,
                        accum_out=sumexp[:, np2, mi:mi + 1],
                    )
                # ship this block pair's sums while the next pair computes
                # (contiguous per-partition run; sync queue is idle by now)
                nc.sync.dma_start(out=out_d[:, np2], in_=sumexp[:, np2])

    nc.compile()
    return nc


def _prep(x: np.ndarray, y: np.ndarray):
    """Host prep: normalize, fp8-quantize, PE-layout both operands."""
    x = np.ascontiguousarray(x, dtype=np.float32)
    y = np.ascontiguousarray(y, dtype=np.float32)
    xn = x / np.maximum(np.linalg.norm(x, axis=1, keepdims=True), EPS)
    yn = y / np.maximum(np.linalg.norm(y, axis=1, keepdims=True), EPS)
    diag = np.einsum("ij,ij->i", xn.astype(np.float64), yn.astype(np.float64))

    f8 = ml_dtypes.float8_e4m3
    xq = (xn * FP8_SCALE).astype(f8)
    yq = (yn * FP8_SCALE).astype(f8)

    # xT[c, mi, p, kb, i, m] = xq[c*1024 + mi*128 + m, kb*256 + i*128 + p]
    xT = np.ascontiguousarray(
        xq.reshape(N_CORES, MT, 128, KB, 2, 128).transpose(0, 1, 5, 3, 4, 2)
    )
    # yT[nb, kb, p, i, n] = yq[nb*1024 + n, kb*256 + i*128 + p]
    yT = np.ascontiguousarray(
        yq.reshape(NB, YBLK, KB, 2, 128).transpose(0, 2, 4, 3, 1)
    )
    return xT, yT, diag


def _finalize(res, diag) -> np.ndarray:
    total = 0.0
    for c in range(N_CORES):
        S = res.results[c]["out"].astype(np.float64).sum(axis=1)  # [p, mi]
        dg = diag[c * ROWS:(c + 1) * ROWS].reshape(MT, 128)       # [mi, p]
        total += (np.log(S.T) - dg / TAU).sum()
    return np.float32(total / B)


def kernel(x: np.ndarray, y: np.ndarray) -> np.ndarray:
    global _compiled
    if _compiled is None:
        _compiled = _build()
    nc = _compiled

    xT, yT, diag = _prep(x, y)
    in_maps = [{"xT": xT[c], "yT": yT} for c in range(N_CORES)]
    res = run_bass_kernel_spmd(nc, in_maps, core_ids=list(range(N_CORES)))
    return _finalize(res, diag)
